# revision 22
# baseline (speedup 1.0000x reference)
"""Trainium2 Bass kernel for nn_FDM_46394236731667.

Computes, per batch b (b = 0..7, one NeuronCore each):
    f1,f2,f3 = fm{1,2,3}[b].reshape(C, HW)
    qn  = f1 / ||f1||_col  (column-wise L2 over channels)
    s_k = -(qn^T @ (f_k / ||f_k||_col))          k in {2,3}
    a_k = softmax(s_k, axis=-1)
    out[b] = f1 + 0.001 * (f2 @ a2^T + f3 @ a3^T)

v2 structure (vs v1):
  - Two single-mat phases: all of mat2 (scores -> exp -> softmax-denom ->
    values, partial results t2 parked in SBUF as bf16), then mat3 with a fused
    epilogue o = f1 + t2 + t3 and a single output DMA per tile. This removes
    the DRAM DMA-accumulate and lets f3's DMA/squares/transposes hide entirely
    under the mat2 compute.
  - Value matmuls loop (i, jj) so each fT weight load serves 2 matmuls into a
    2-bank psum tile; halves LDWEIGHTS pressure (LDWEIGHTS is the dominant
    un-modeled HW cost for DoubleRow streams).
  - All column-norm reciprocals via exp(-0.5*ln(x)+b) so only the
    natural_log_exp ACT table set is ever loaded (no mid-stream table
    switches); the query-norm sign folds into qn via scalar_tensor_tensor.
  - PE-transpose PSUM evacuations batched 8 per DVE copy.

Numerics as v1: scores/denominator/value matmuls in fp8(e4m3) DoubleRow; the
output is fm1 + 0.001*(attention terms) so low-precision attention arithmetic
perturbs the output by ~1e-6 relative. Softmax skips max-subtraction: scores
are cosine similarities in [-1, 1] (times 1/16 fp8 headroom scale folded into
the exp scale), so exp() cannot overflow.
"""
import math
import os
import sys

for _p in ("/opt/trn_rl_repo", "/root/.axon_site/_ro/trn_rl_repo"):
    if os.path.isdir(_p) and _p not in sys.path:
        sys.path.insert(0, _p)

import numpy as np

import concourse.bass as bass
import concourse.tile as tile
from concourse import bacc, mybir
from concourse.bass_utils import run_bass_kernel_spmd
from concourse.masks import make_identity

B, C, H, W = 8, 512, 56, 56
HW = H * W            # 3136
P = 128
CC = C // P           # 4 channel chunks
NMC = 25              # m chunks: 24 x 128 + 1 x 64
MTAIL = HW - 24 * P   # 64
NPAIR = NMC // 2      # 12 DoubleRow m-chunk pairs (+1 tail chunk)
NNC = 7               # n chunks
NW = HW // NNC        # 448
NJP = (NNC + 1) // 2  # 4 n-chunk pairs: (0,1),(2,3),(4,5),(6,)
FACTOR = 0.001
QSCALE = 16.0         # fp8 headroom scale on qn; 1/QSCALE folded into exp
LNQ = math.log(QSCALE)

dt = mybir.dt
F32, BF16, FP8 = dt.float32, dt.bfloat16, dt.float8e4
E3 = dt.float8e3
DR = mybir.MatmulPerfMode.DoubleRow
AF = mybir.ActivationFunctionType
MUL = mybir.AluOpType.mult

TRACE = False
_cached_nc = None


def _mw(mc):
    return P if mc < NMC - 1 else MTAIL


def _npj(jp):
    return 2 if 2 * jp + 1 < NNC else 1


def _jss(jp):
    return [slice((2 * jp + jj) * NW, (2 * jp + jj + 1) * NW)
            for jj in range(_npj(jp))]


def _quake_rsqrt(nc, pool, ps_src, out, postscale, name):
    """out = postscale / sqrt(ps_src), computed entirely on DVE (bit-trick
    seed + 2 Newton iterations, ~5e-6 max rel err). Keeps the mid-exp-stream
    norm off ACT so the exp table set is never evicted."""
    shp = list(out.shape)
    MAGIC = 0x5F3759DF
    a = pool.tile(shp, dt.int32, tag="qr", bufs=8, name=f"{name}_a")
    nc.vector.tensor_scalar(a, ps_src.bitcast(dt.int32), 1, None,
                            op0=mybir.AluOpType.logical_shift_right)
    b = pool.tile(shp, dt.int32, tag="qr", bufs=8, name=f"{name}_b")
    nc.vector.tensor_scalar(b, a, -1, MAGIC, op0=MUL,
                            op1=mybir.AluOpType.add)
    xs = pool.tile(shp, F32, tag="qr", bufs=8, name=f"{name}_x")
    nc.vector.tensor_copy(xs, ps_src)
    y = b.bitcast(F32)
    for it in range(2):
        t = pool.tile(shp, F32, tag="qr", bufs=8, name=f"{name}_t{it}")
        nc.vector.tensor_mul(t, xs, y)
        u = pool.tile(shp, F32, tag="qr", bufs=8, name=f"{name}_u{it}")
        nc.vector.tensor_mul(u, t, y)
        v = pool.tile(shp, F32, tag="qr", bufs=8, name=f"{name}_v{it}")
        nc.vector.tensor_scalar(v, u, -0.5, 1.5, op0=MUL,
                                op1=mybir.AluOpType.add)
        w = pool.tile(shp, F32, tag="qr", bufs=8, name=f"{name}_w{it}")
        nc.vector.tensor_mul(w, y, v)
        y = w
    nc.vector.tensor_scalar_mul(out, y, postscale)


def _build_preproc(tc, sbP, pre, ps, fm1, fm2, st):
    """f1 -> qn (negated, scaled fp8), f2 -> fb2 + rpos2. Transient tiles in
    `pre` (released before the main pool opens)."""
    nc = tc.nc

    # fp8e5-view identity for PE transposes of e4m3 data (pure data movement)
    ident = sbP.tile([P, P], E3, tag="ident", name="ident")
    make_identity(nc, ident)
    ones128 = sbP.tile([P, 2, P], FP8, tag="ones128", name="ones128")
    nc.vector.memset(ones128, 1.0)
    ones_col = sbP.tile([P, 1], FP8, tag="ones_col", name="ones_col")
    nc.vector.memset(ones_col, 1.0)

    # ---- f1: squares, column norms, qn = -(QSCALE/||f1||) * f1 in fp8 ----
    # DMA quarter-granular and h-major across channel chunks so the first
    # n-chunks' norms (and thus qn and the first score matmuls) unblock at
    # ~1/4 of the f1 DMA latency instead of waiting for all of f1.
    fsq1 = [pre.tile([P, HW], FP8, tag="fsq", bufs=8, name=f"fsq1_{cc}")
            for cc in range(CC)]
    fr1b = [pre.tile([P, HW], F32, tag="fraw", bufs=8, name=f"f1raw_{cc}")
            for cc in range(CC)]
    for h in range(4):
        hs = slice(h * (HW // 4), (h + 1) * (HW // 4))
        for cc in range(CC):
            nc.sync.dma_start(fr1b[cc][:, hs], fm1[cc * P:(cc + 1) * P, hs])
            # squares on DVE: keeps ACT free for the exp stream ramp
            nc.vector.tensor_mul(fsq1[cc][:, hs], fr1b[cc][:, hs],
                                 fr1b[cc][:, hs])

    rbf = pre.tile([P, HW], F32, tag="rbf", name="rbf")
    qn = sbP.tile([P, CC, HW], FP8, tag="qn", name="qn")
    for j in range(NNC):
        js = slice(j * NW, (j + 1) * NW)
        ssb = ps.tile([P, NW], F32, tag="vp", bufs=2, name=f"ssb_{j}")
        for cc in range(CC):
            nc.tensor.matmul(ssb, ones128[:, 0, :], fsq1[cc][:, js],
                             start=(cc == 0), stop=(cc == CC - 1))
        # rbf = QSCALE/||f1col||, broadcast over partitions
        ns = pre.tile([P, NW], F32, tag="rtmp", bufs=2, name=f"ns1_{j}")
        nc.scalar.activation(ns, ssb, AF.Sqrt, bias=0.0,
                             scale=1.0 / (QSCALE * QSCALE))
        nc.vector.reciprocal_approx_fast(rbf[:, js], ns)
        for cc in range(CC):
            # qn = f1 * rbf on Pool (DVE is the ramp bottleneck, Pool idle
            # until f3 lands). The score negation rides on rpos2/rpos3.
            nc.gpsimd.tensor_mul(qn[:, cc, js], fr1b[cc][:, js], rbf[:, js])

    # ---- f2: fp8 copy + per-m-column norms rpos2 = 1/(QSCALE*||f2col||) ----
    fb2 = sbP.tile([P, CC, HW], FP8, tag="k2b", name="k2b")
    fsq2 = []
    for cc in range(CC):
        fr = pre.tile([P, HW], F32, tag="fraw", bufs=8, name=f"k2raw_{cc}")
        t8 = pre.tile([P, HW], FP8, tag="fsq", bufs=8, name=f"k2sq_{cc}")
        for h in range(2):
            hs = slice(h * (HW // 2), (h + 1) * (HW // 2))
            nc.sync.dma_start(fr[:, hs], fm2[cc * P:(cc + 1) * P, hs])
            nc.vector.tensor_copy(fb2[:, cc, hs], fr[:, hs])
            nc.scalar.square(t8[:, hs], fr[:, hs])
        fsq2.append(t8)

    # rpos2 in 4 mc-ranges with one psum tile per range: PSUM dependency
    # tracking is bank-granular, so a single [P, NMC] tile would gate the
    # first exps on the LAST of f2's DMA quarters instead of the first.
    nrm2 = pre.tile([P, NMC], F32, tag="rtmp2", bufs=2, name="nrm2")
    rpos2 = sbP.tile([P, NMC], F32, tag="rpos2", name="rpos2")
    for a, b in ((0, 6), (6, 12), (12, 18), (18, NMC)):
        ssc2 = ps.tile([P, b - a], F32, tag="vp", bufs=2, name=f"ssc2_{a}")
        for mc in range(a, b):
            mw = _mw(mc)
            msl = slice(mc * P, mc * P + mw)
            for cc in range(CC):
                nc.tensor.matmul(ssc2[:mw, mc - a:mc - a + 1],
                                 fsq2[cc][:, msl], ones_col,
                                 start=(cc == 0), stop=(cc == CC - 1))
        nc.scalar.activation(nrm2[:, a:b], ssc2, AF.Sqrt, bias=0.0,
                             scale=QSCALE * QSCALE)
        nc.vector.reciprocal_approx_fast(rpos2[:, a:b], nrm2[:, a:b])
        nc.vector.tensor_scalar_mul(rpos2[:, a:b], rpos2[:, a:b], -1.0)

    st.update(ident=ident, ones128=ones128, ones_col=ones_col, qn=qn,
              fb2=fb2, rpos2=rpos2)


def _f3_load(tc, sbP, sbm, fm3):
    """f3 DMA + squares + fp8 copy. Emitted at the top of the main region so
    the DMAs queue right behind f2's. The elementwise work runs on the idle
    Pool engine: by the time f3 lands, ACT is saturated by the exp stream and
    DVE by fT evacuations + value epilogues, so putting these there (at high
    program-order priority) would stall the critical path."""
    nc = tc.nc
    fb3 = sbP.tile([P, CC, HW], FP8, tag="k3b", name="k3b")
    fsq3 = []
    for cc in range(CC):
        t8 = sbm.tile([P, HW], FP8, tag="k3sq", bufs=4, name=f"k3sq_{cc}")
        for h in range(2):
            hs = slice(h * (HW // 2), (h + 1) * (HW // 2))
            fr = sbm.tile([P, HW // 2], F32, tag="k3raw", bufs=2,
                          name=f"k3raw_{cc}_{h}")
            nc.sync.dma_start(fr, fm3[cc * P:(cc + 1) * P, hs])
            nc.gpsimd.tensor_copy(fb3[:, cc, hs], fr)
            nc.gpsimd.tensor_mul(t8[:, hs], fr, fr)
        fsq3.append(t8)
    return fb3, fsq3


def _f3_norms(tc, sbP, sbm, ps, st, fsq3):
    """Column norms of f3. Emitted mid-mat2 so its psum slot isn't reserved
    while f3's DMA is still in flight."""
    nc = tc.nc
    ones_col = st["ones_col"]
    ssc3 = ps.tile([P, NMC], F32, tag="vp", bufs=2, name="ssc3")
    for mc in range(NMC):
        mw = _mw(mc)
        msl = slice(mc * P, mc * P + mw)
        for cc in range(CC):
            nc.tensor.matmul(ssc3[:mw, mc:mc + 1], fsq3[cc][:, msl], ones_col,
                             start=(cc == 0), stop=(cc == CC - 1))
    rpos3 = sbP.tile([P, NMC], F32, tag="rpos3", name="rpos3")
    _quake_rsqrt(nc, sbm, ssc3, rpos3, -1.0 / QSCALE, "qr3")
    st["rpos3"] = rpos3


def _build_main(tc, sbP, sbm, ps, out_ap, fm1, fm3, st):
    nc = tc.nc
    qn = st["qn"]
    ones128 = st["ones128"]
    ident = st["ident"]

    fb3, fsq3 = _f3_load(tc, sbP, sbm, fm3)
    fT2 = sbP.tile([P, NMC, C], FP8, tag="k2T", name="k2T")
    fT3 = sbP.tile([P, NMC, C], FP8, tag="k3T", name="k3T")

    # ---- transpose machinery: fT[p, mc, c] = f[c, mc*128+p] ----
    # 8 PE transposes land in one 1-bank psum tile; a single DVE copy
    # evacuates all 8 (batched to amortize DVE op overhead).
    def tp_group(fT, fb, label, cc, mc0, k):
        tp = ps.tile([P, 8, P, 2], E3, tag="vp", bufs=2,
                     name=f"tp_{label}_{cc}_{mc0}")
        mw = _mw(mc0 + k - 1)  # tail only ever alone in a group
        for g in range(k):
            mc = mc0 + g
            msl = slice(mc * P, mc * P + _mw(mc))
            nc.tensor.transpose(tp[:_mw(mc), g, :, 0],
                                fb[:, cc, msl].bitcast(E3), ident)
        nc.vector.tensor_copy(
            fT[:mw, mc0:mc0 + k, cc * P:(cc + 1) * P].bitcast(E3),
            tp[:mw, :k, :, 0])

    def tp_jobs(fT, fb, label):
        jobs = []
        for cc in range(CC):
            for g in range(3):
                jobs.append((fT, fb, label, cc, 8 * g, 8))
            jobs.append((fT, fb, label, cc, 24, 1))
        return jobs

    def make_drip(jobs):
        it = iter(jobs)

        def drip(n=1):
            for _ in range(n):
                j = next(it, None)
                if j is None:
                    return
                tp_group(*j)
        return drip

    drip2 = make_drip(tp_jobs(fT2, st["fb2"], "k2"))
    drip3 = make_drip(tp_jobs(fT3, fb3, "k3"))

    # ---- score + exp emission for one (mat, n-chunk-pair) ----
    Es = {}

    def emit_scores(mat, jp, drip=None):
        fb = st["fb2"] if mat == 2 else fb3
        rpos = st["rpos2"] if mat == 2 else st["rpos3"]
        npj = _npj(jp)
        jss = _jss(jp)
        E = sbm.tile([P, NMC, 2, NW], FP8, tag="E", bufs=3,
                     name=f"E{mat}_{jp}")
        Es[(mat, jp)] = E
        for mc in range(NMC):
            mw = _mw(mc)
            msl = slice(mc * P, mc * P + mw)
            # [128, 1024] spans 2 psum banks; halves at 0 and 512 so each
            # matmul output stays inside one bank
            sp = ps.tile([P, 1024], F32, tag="sp", bufs=2,
                         name=f"sp_{mat}_{jp}_{mc}")
            for i in range(CC // 2):
                for jj in range(npj):
                    nc.tensor.matmul(sp[:mw, jj * 512:jj * 512 + NW],
                                     fb[:, 2 * i:2 * i + 2, msl],
                                     qn[:, 2 * i:2 * i + 2, jss[jj]],
                                     start=(i == 0), stop=(i == CC // 2 - 1),
                                     perf_mode=DR)
            spv = sp[:mw, :].rearrange("p (t x) -> p t x", t=2)
            nc.scalar.activation(E[:mw, mc, :npj, :], spv[:, :npj, :NW],
                                 AF.Exp, bias=0.0, scale=rpos[:mw, mc:mc + 1])
            if drip is not None:
                drip(1)
        if drip is not None:
            drip(100)  # flush leftovers

    # ---- softmax denominator + value matmuls for one (mat, pair) ----
    T2 = {}

    def values(mat, jp):
        E = Es.pop((mat, jp))
        fT = fT2 if mat == 2 else fT3
        npj = _npj(jp)
        jss = _jss(jp)
        rss = []
        for jj in range(npj):
            cs = ps.tile([P, NW], F32, tag="vp", bufs=2,
                         name=f"cs_{mat}_{jp}_{jj}")
            for i in range(NPAIR):
                nc.tensor.matmul(cs, ones128, E[:, 2 * i:2 * i + 2, jj, :],
                                 start=(i == 0), stop=False, perf_mode=DR)
            nc.tensor.matmul(cs, ones128[:MTAIL, 0, :],
                             E[:MTAIL, NMC - 1, jj, :],
                             start=False, stop=True)
            rs = sbm.tile([P, NW], F32, tag="rs", bufs=4,
                          name=f"rs_{mat}_{jp}_{jj}")
            nc.vector.reciprocal_approx_fast(rs, cs)
            if mat == 3:
                nc.vector.tensor_scalar_mul(rs, rs, FACTOR)
            rss.append(rs)
        for cc in range(CC):
            csl = slice(cc * P, (cc + 1) * P)
            vp = ps.tile([P, 1024], F32, tag="vp", bufs=2,
                         name=f"vp_{mat}_{jp}_{cc}")
            for i in range(NPAIR):
                for jj in range(npj):
                    nc.tensor.matmul(vp[:, jj * 512:jj * 512 + NW],
                                     fT[:, 2 * i:2 * i + 2, csl],
                                     E[:, 2 * i:2 * i + 2, jj, :],
                                     start=(i == 0), stop=False, perf_mode=DR)
            for jj in range(npj):
                nc.tensor.matmul(vp[:, jj * 512:jj * 512 + NW],
                                 fT[:MTAIL, NMC - 1, csl],
                                 E[:MTAIL, NMC - 1, jj, :],
                                 start=False, stop=True)
            for jj in range(npj):
                vslice = vp[:, jj * 512:jj * 512 + NW]
                if mat == 2:
                    # park FACTOR * (f2 @ a2^T) in bf16 until the mat3 phase
                    t2 = sbm.tile([P, NW], BF16, tag="t2", bufs=28,
                                  name=f"t2_{jp}_{jj}_{cc}")
                    nc.vector.scalar_tensor_tensor(t2, vslice, FACTOR, rss[jj],
                                                   MUL, MUL)
                    T2[(jp, jj, cc)] = t2
                else:
                    js = jss[jj]
                    ta = sbm.tile([P, NW], F32, tag="t", bufs=4,
                                  name=f"ta_{jp}_{jj}_{cc}")
                    nc.vector.tensor_mul(ta, vslice, rss[jj])
                    if jp == NJP - 1:
                        eng = nc.vector if cc % 2 == 0 else nc.gpsimd
                    else:
                        eng = nc.gpsimd
                    tb = sbm.tile([P, NW], F32, tag="t", bufs=4,
                                  name=f"tb_{jp}_{jj}_{cc}")
                    eng.tensor_add(tb, ta, T2.pop((jp, jj, cc)))
                    fs = sbm.tile([P, NW], F32, tag="f1s", bufs=4,
                                  name=f"f1s_{jp}_{jj}_{cc}")
                    nc.sync.dma_start(fs, fm1[csl, js])
                    o = sbm.tile([P, NW], F32, tag="outs", bufs=4,
                                 name=f"o_{jp}_{jj}_{cc}")
                    eng.tensor_add(o, tb, fs)
                    nc.sync.dma_start(out_ap[csl, js], o)

    # ---- software pipeline: scores for the next pair are emitted (and thus
    # PE-prioritized) ahead of the value phase of the current pair ----
    emit_scores(2, 0, drip=drip2)
    emit_scores(2, 1)
    emit_scores(2, 2)
    values(2, 0)
    emit_scores(2, 3, drip=drip3)
    values(2, 1)
    _f3_norms(tc, sbP, sbm, ps, st, fsq3)
    emit_scores(3, 0)
    values(2, 2)
    emit_scores(3, 1)
    values(2, 3)
    emit_scores(3, 2)
    values(3, 0)
    emit_scores(3, 3)
    values(3, 1)
    values(3, 2)
    values(3, 3)


def _build():
    nc = bacc.Bacc("TRN2", target_bir_lowering=False, debug=False,
                   num_devices=B)
    fm1 = nc.dram_tensor("fm1", [C, HW], F32, kind="ExternalInput").ap()
    fm2 = nc.dram_tensor("fm2", [C, HW], F32, kind="ExternalInput").ap()
    fm3 = nc.dram_tensor("fm3", [C, HW], F32, kind="ExternalInput").ap()
    out = nc.dram_tensor("out", [C, HW], F32, kind="ExternalOutput").ap()

    with tile.TileContext(nc) as tc:
        with tc.tile_pool(name="sbP", bufs=1) as sbP, \
             tc.tile_pool(name="ps", bufs=1, space="PSUM") as ps:
            st = {}
            with tc.tile_pool(name="pre", bufs=1) as pre:
                _build_preproc(tc, sbP, pre, ps, fm1, fm2, st)
            with tc.tile_pool(name="sbm", bufs=1) as sbm:
                _build_main(tc, sbP, sbm, ps, out, fm1, fm3, st)
    nc.compile()
    return nc


def _get_nc():
    global _cached_nc
    if _cached_nc is None:
        _cached_nc = _build()
    return _cached_nc


def kernel(**inputs):
    fm1 = np.ascontiguousarray(
        np.asarray(inputs["fm1"], dtype=np.float32).reshape(B, C, HW))
    fm2 = np.ascontiguousarray(
        np.asarray(inputs["fm2"], dtype=np.float32).reshape(B, C, HW))
    fm3 = np.ascontiguousarray(
        np.asarray(inputs["fm3"], dtype=np.float32).reshape(B, C, HW))

    nc = _get_nc()
    in_maps = [{"fm1": fm1[b], "fm2": fm2[b], "fm3": fm3[b]} for b in range(B)]
    res = run_bass_kernel_spmd(nc, in_maps, core_ids=list(range(B)),
                               trace=TRACE)
    kernel.last_results = res
    out = np.stack([res.results[b]["out"] for b in range(B)])
    return out.reshape(B, C, H, W).astype(np.float32)


kernel.last_results = None


if __name__ == "__main__":
    rng = np.random.default_rng(0)
    ins = {k: rng.standard_normal((B, C, H, W)).astype(np.float32)
           for k in ("fm1", "fm2", "fm3")}
    o = kernel(**ins)
    print("out shape", o.shape, o.dtype)


# revision 23
# speedup vs baseline: 11.4581x; 11.4581x over previous
"""Trainium2 Bass kernel for nn_FDM_46394236731667.

Computes, per batch b (b = 0..7, one NeuronCore each):
    f1,f2,f3 = fm{1,2,3}[b].reshape(C, HW)
    qn  = f1 / ||f1||_col  (column-wise L2 over channels)
    s_k = -(qn^T @ (f_k / ||f_k||_col))          k in {2,3}
    a_k = softmax(s_k, axis=-1)
    out[b] = f1 + 0.001 * (f2 @ a2^T + f3 @ a3^T)

v2 structure (vs v1):
  - Two single-mat phases: all of mat2 (scores -> exp -> softmax-denom ->
    values, partial results t2 parked in SBUF as bf16), then mat3 with a fused
    epilogue o = f1 + t2 + t3 and a single output DMA per tile. This removes
    the DRAM DMA-accumulate and lets f3's DMA/squares/transposes hide entirely
    under the mat2 compute.
  - Value matmuls loop (i, jj) so each fT weight load serves 2 matmuls into a
    2-bank psum tile; halves LDWEIGHTS pressure (LDWEIGHTS is the dominant
    un-modeled HW cost for DoubleRow streams).
  - All column-norm reciprocals via exp(-0.5*ln(x)+b) so only the
    natural_log_exp ACT table set is ever loaded (no mid-stream table
    switches); the query-norm sign folds into qn via scalar_tensor_tensor.
  - PE-transpose PSUM evacuations batched 8 per DVE copy.

Numerics as v1: scores/denominator/value matmuls in fp8(e4m3) DoubleRow; the
output is fm1 + 0.001*(attention terms) so low-precision attention arithmetic
perturbs the output by ~1e-6 relative. Softmax skips max-subtraction: scores
are cosine similarities in [-1, 1] (times 1/16 fp8 headroom scale folded into
the exp scale), so exp() cannot overflow.
"""
import math
import os
import sys

for _p in ("/opt/trn_rl_repo", "/root/.axon_site/_ro/trn_rl_repo"):
    if os.path.isdir(_p) and _p not in sys.path:
        sys.path.insert(0, _p)

import numpy as np

import concourse.bass as bass
import concourse.tile as tile
from concourse import bacc, mybir
from concourse.bass_utils import run_bass_kernel_spmd
from concourse.masks import make_identity

B, C, H, W = 8, 512, 56, 56
HW = H * W            # 3136
P = 128
CC = C // P           # 4 channel chunks
NMC = 25              # m chunks: 24 x 128 + 1 x 64
MTAIL = HW - 24 * P   # 64
NPAIR = NMC // 2      # 12 DoubleRow m-chunk pairs (+1 tail chunk)
NNC = 7               # n chunks
NW = HW // NNC        # 448
NJP = (NNC + 1) // 2  # 4 n-chunk pairs: (0,1),(2,3),(4,5),(6,)
FACTOR = 0.001
QSCALE = 16.0         # fp8 headroom scale on qn; 1/QSCALE folded into exp
LNQ = math.log(QSCALE)

dt = mybir.dt
F32, BF16, FP8 = dt.float32, dt.bfloat16, dt.float8e4
E3 = dt.float8e3
DR = mybir.MatmulPerfMode.DoubleRow
AF = mybir.ActivationFunctionType
MUL = mybir.AluOpType.mult

TRACE = False
_cached_nc = None


def _mw(mc):
    return P if mc < NMC - 1 else MTAIL


def _npj(jp):
    return 2 if 2 * jp + 1 < NNC else 1


def _jss(jp):
    return [slice((2 * jp + jj) * NW, (2 * jp + jj + 1) * NW)
            for jj in range(_npj(jp))]


def _quake_rsqrt(nc, pool, ps_src, out, postscale, name):
    """out = postscale / sqrt(ps_src), computed entirely on DVE (bit-trick
    seed + 2 Newton iterations, ~5e-6 max rel err). Keeps the mid-exp-stream
    norm off ACT so the exp table set is never evicted."""
    shp = list(out.shape)
    MAGIC = 0x5F3759DF
    a = pool.tile(shp, dt.int32, tag="qr", bufs=8, name=f"{name}_a")
    nc.vector.tensor_scalar(a, ps_src.bitcast(dt.int32), 1, None,
                            op0=mybir.AluOpType.logical_shift_right)
    b = pool.tile(shp, dt.int32, tag="qr", bufs=8, name=f"{name}_b")
    nc.vector.tensor_scalar(b, a, -1, MAGIC, op0=MUL,
                            op1=mybir.AluOpType.add)
    xs = pool.tile(shp, F32, tag="qr", bufs=8, name=f"{name}_x")
    nc.vector.tensor_copy(xs, ps_src)
    y = b.bitcast(F32)
    for it in range(2):
        t = pool.tile(shp, F32, tag="qr", bufs=8, name=f"{name}_t{it}")
        nc.vector.tensor_mul(t, xs, y)
        u = pool.tile(shp, F32, tag="qr", bufs=8, name=f"{name}_u{it}")
        nc.vector.tensor_mul(u, t, y)
        v = pool.tile(shp, F32, tag="qr", bufs=8, name=f"{name}_v{it}")
        nc.vector.tensor_scalar(v, u, -0.5, 1.5, op0=MUL,
                                op1=mybir.AluOpType.add)
        w = pool.tile(shp, F32, tag="qr", bufs=8, name=f"{name}_w{it}")
        nc.vector.tensor_mul(w, y, v)
        y = w
    nc.vector.tensor_scalar_mul(out, y, postscale)


def _build_preproc(tc, sbP, pre, ps, fm1, fm2, st):
    """f1 -> qn (negated, scaled fp8), f2 -> fb2 + rpos2. Transient tiles in
    `pre` (released before the main pool opens)."""
    nc = tc.nc

    # fp8e5-view identity for PE transposes of e4m3 data (pure data movement)
    ident = sbP.tile([P, P], E3, tag="ident", name="ident")
    make_identity(nc, ident)
    ones128 = sbP.tile([P, 2, P], FP8, tag="ones128", name="ones128")
    nc.vector.memset(ones128, 1.0)
    ones_col = sbP.tile([P, 1], FP8, tag="ones_col", name="ones_col")
    nc.vector.memset(ones_col, 1.0)

    # ---- f1: squares, column norms, qn = -(QSCALE/||f1||) * f1 in fp8 ----
    # DMA quarter-granular and h-major across channel chunks so the first
    # n-chunks' norms (and thus qn and the first score matmuls) unblock at
    # ~1/4 of the f1 DMA latency instead of waiting for all of f1.
    fsq1 = [pre.tile([P, HW], FP8, tag="fsq", bufs=8, name=f"fsq1_{cc}")
            for cc in range(CC)]
    fr1b = [pre.tile([P, HW], F32, tag="fraw", bufs=8, name=f"f1raw_{cc}")
            for cc in range(CC)]
    for h in range(4):
        hs = slice(h * (HW // 4), (h + 1) * (HW // 4))
        for cc in range(CC):
            nc.sync.dma_start(fr1b[cc][:, hs], fm1[cc * P:(cc + 1) * P, hs])
            # squares on DVE: keeps ACT free for the exp stream ramp
            nc.vector.tensor_mul(fsq1[cc][:, hs], fr1b[cc][:, hs],
                                 fr1b[cc][:, hs])

    rbf = pre.tile([P, HW], F32, tag="rbf", name="rbf")
    qn = sbP.tile([P, CC, HW], FP8, tag="qn", name="qn")
    for j in range(NNC):
        js = slice(j * NW, (j + 1) * NW)
        ssb = ps.tile([P, NW], F32, tag="vp", bufs=2, name=f"ssb_{j}")
        for cc in range(CC):
            nc.tensor.matmul(ssb, ones128[:, 0, :], fsq1[cc][:, js],
                             start=(cc == 0), stop=(cc == CC - 1))
        # rbf = QSCALE/||f1col||, broadcast over partitions
        ns = pre.tile([P, NW], F32, tag="rtmp", bufs=2, name=f"ns1_{j}")
        nc.scalar.activation(ns, ssb, AF.Sqrt, bias=0.0,
                             scale=1.0 / (QSCALE * QSCALE))
        nc.vector.reciprocal_approx_fast(rbf[:, js], ns)
        for cc in range(CC):
            # qn = f1 * rbf on Pool (DVE is the ramp bottleneck, Pool idle
            # until f3 lands). The score negation rides on rpos2/rpos3.
            nc.gpsimd.tensor_mul(qn[:, cc, js], fr1b[cc][:, js], rbf[:, js])

    # ---- f2: fp8 copy + per-m-column norms rpos2 = 1/(QSCALE*||f2col||) ----
    fb2 = sbP.tile([P, CC, HW], FP8, tag="k2b", name="k2b")
    fsq2 = []
    for cc in range(CC):
        fr = pre.tile([P, HW], F32, tag="fraw", bufs=8, name=f"k2raw_{cc}")
        t8 = pre.tile([P, HW], FP8, tag="fsq", bufs=8, name=f"k2sq_{cc}")
        for h in range(2):
            hs = slice(h * (HW // 2), (h + 1) * (HW // 2))
            nc.sync.dma_start(fr[:, hs], fm2[cc * P:(cc + 1) * P, hs])
            nc.vector.tensor_copy(fb2[:, cc, hs], fr[:, hs])
            nc.scalar.square(t8[:, hs], fr[:, hs])
        fsq2.append(t8)

    # rpos2 in 4 mc-ranges with one psum tile per range: PSUM dependency
    # tracking is bank-granular, so a single [P, NMC] tile would gate the
    # first exps on the LAST of f2's DMA quarters instead of the first.
    nrm2 = pre.tile([P, NMC], F32, tag="rtmp2", bufs=2, name="nrm2")
    rpos2 = sbP.tile([P, NMC], F32, tag="rpos2", name="rpos2")
    for a, b in ((0, 6), (6, 12), (12, 18), (18, NMC)):
        ssc2 = ps.tile([P, b - a], F32, tag="vp", bufs=2, name=f"ssc2_{a}")
        for mc in range(a, b):
            mw = _mw(mc)
            msl = slice(mc * P, mc * P + mw)
            for cc in range(CC):
                nc.tensor.matmul(ssc2[:mw, mc - a:mc - a + 1],
                                 fsq2[cc][:, msl], ones_col,
                                 start=(cc == 0), stop=(cc == CC - 1))
        nc.scalar.activation(nrm2[:, a:b], ssc2, AF.Sqrt, bias=0.0,
                             scale=QSCALE * QSCALE)
        nc.vector.reciprocal_approx_fast(rpos2[:, a:b], nrm2[:, a:b])
        nc.vector.tensor_scalar_mul(rpos2[:, a:b], rpos2[:, a:b], -1.0)

    st.update(ident=ident, ones128=ones128, ones_col=ones_col, qn=qn,
              fb2=fb2, rpos2=rpos2)


def _f3_load(tc, sbP, sbm, fm3):
    """f3 DMA + squares + fp8 copy. Emitted at the top of the main region so
    the DMAs queue right behind f2's. The elementwise work runs on the idle
    Pool engine: by the time f3 lands, ACT is saturated by the exp stream and
    DVE by fT evacuations + value epilogues, so putting these there (at high
    program-order priority) would stall the critical path."""
    nc = tc.nc
    fb3 = sbP.tile([P, CC, HW], FP8, tag="k3b", name="k3b")
    fsq3 = []
    for cc in range(CC):
        t8 = sbm.tile([P, HW], FP8, tag="k3sq", bufs=4, name=f"k3sq_{cc}")
        for h in range(2):
            hs = slice(h * (HW // 2), (h + 1) * (HW // 2))
            fr = sbm.tile([P, HW // 2], F32, tag="k3raw", bufs=2,
                          name=f"k3raw_{cc}_{h}")
            nc.sync.dma_start(fr, fm3[cc * P:(cc + 1) * P, hs])
            nc.gpsimd.tensor_copy(fb3[:, cc, hs], fr)
            nc.gpsimd.tensor_mul(t8[:, hs], fr, fr)
        fsq3.append(t8)
    return fb3, fsq3


def _f3_norms(tc, sbP, sbm, ps, st, fsq3):
    """Column norms of f3. Emitted mid-mat2 so its psum slot isn't reserved
    while f3's DMA is still in flight."""
    nc = tc.nc
    ones_col = st["ones_col"]
    ssc3 = ps.tile([P, NMC], F32, tag="vp", bufs=2, name="ssc3")
    for mc in range(NMC):
        mw = _mw(mc)
        msl = slice(mc * P, mc * P + mw)
        for cc in range(CC):
            nc.tensor.matmul(ssc3[:mw, mc:mc + 1], fsq3[cc][:, msl], ones_col,
                             start=(cc == 0), stop=(cc == CC - 1))
    rpos3 = sbP.tile([P, NMC], F32, tag="rpos3", name="rpos3")
    _quake_rsqrt(nc, sbm, ssc3, rpos3, -1.0 / QSCALE, "qr3")
    st["rpos3"] = rpos3


def _build_main(tc, sbP, sbm, ps, out_ap, fm1, fm3, st):
    nc = tc.nc
    qn = st["qn"]
    ones128 = st["ones128"]
    ident = st["ident"]

    fb3, fsq3 = _f3_load(tc, sbP, sbm, fm3)
    fT2 = sbP.tile([P, NMC, C], FP8, tag="k2T", name="k2T")
    fT3 = sbP.tile([P, NMC, C], FP8, tag="k3T", name="k3T")

    # ---- transpose machinery: fT[p, mc, c] = f[c, mc*128+p] ----
    # 8 PE transposes land in one 1-bank psum tile; a single DVE copy
    # evacuates all 8 (batched to amortize DVE op overhead).
    def tp_group(fT, fb, label, cc, mc0, k):
        tp = ps.tile([P, 8, P, 2], E3, tag="vp", bufs=2,
                     name=f"tp_{label}_{cc}_{mc0}")
        mw = _mw(mc0 + k - 1)  # tail only ever alone in a group
        for g in range(k):
            mc = mc0 + g
            msl = slice(mc * P, mc * P + _mw(mc))
            nc.tensor.transpose(tp[:_mw(mc), g, :, 0],
                                fb[:, cc, msl].bitcast(E3), ident)
        nc.vector.tensor_copy(
            fT[:mw, mc0:mc0 + k, cc * P:(cc + 1) * P].bitcast(E3),
            tp[:mw, :k, :, 0])

    def tp_jobs(fT, fb, label):
        jobs = []
        for cc in range(CC):
            for g in range(3):
                jobs.append((fT, fb, label, cc, 8 * g, 8))
            jobs.append((fT, fb, label, cc, 24, 1))
        return jobs

    def make_drip(jobs):
        it = iter(jobs)

        def drip(n=1):
            for _ in range(n):
                j = next(it, None)
                if j is None:
                    return
                tp_group(*j)
        return drip

    drip2 = make_drip(tp_jobs(fT2, st["fb2"], "k2"))
    drip3 = make_drip(tp_jobs(fT3, fb3, "k3"))

    # ---- score + exp emission for one (mat, n-chunk-pair) ----
    Es = {}

    def emit_scores(mat, jp, drip=None):
        fb = st["fb2"] if mat == 2 else fb3
        rpos = st["rpos2"] if mat == 2 else st["rpos3"]
        npj = _npj(jp)
        jss = _jss(jp)
        E = sbm.tile([P, NMC, 2, NW], FP8, tag="E", bufs=3,
                     name=f"E{mat}_{jp}")
        Es[(mat, jp)] = E
        for mc in range(NMC):
            mw = _mw(mc)
            msl = slice(mc * P, mc * P + mw)
            # [128, 1024] spans 2 psum banks; halves at 0 and 512 so each
            # matmul output stays inside one bank
            sp = ps.tile([P, 1024], F32, tag="sp", bufs=2,
                         name=f"sp_{mat}_{jp}_{mc}")
            for i in range(CC // 2):
                for jj in range(npj):
                    nc.tensor.matmul(sp[:mw, jj * 512:jj * 512 + NW],
                                     fb[:, 2 * i:2 * i + 2, msl],
                                     qn[:, 2 * i:2 * i + 2, jss[jj]],
                                     start=(i == 0), stop=(i == CC // 2 - 1),
                                     perf_mode=DR)
            spv = sp[:mw, :].rearrange("p (t x) -> p t x", t=2)
            nc.scalar.activation(E[:mw, mc, :npj, :], spv[:, :npj, :NW],
                                 AF.Exp, bias=0.0, scale=rpos[:mw, mc:mc + 1])
            if drip is not None:
                drip(1)
        if drip is not None:
            drip(100)  # flush leftovers

    # ---- softmax denominator + value matmuls for one (mat, pair) ----
    T2 = {}

    def values(mat, jp, steal_sp=False):
        # steal_sp: the drain pair runs after the last exp, so the score
        # psum banks are free -- use both tags for 4-slot accumulation
        E = Es.pop((mat, jp))
        fT = fT2 if mat == 2 else fT3
        npj = _npj(jp)
        jss = _jss(jp)
        tags = ("vp", "sp") if steal_sp else ("vp", "vp")
        rss = []
        for jj in range(npj):
            cs = ps.tile([P, NW], F32, tag=tags[jj % 2], bufs=2,
                         name=f"cs_{mat}_{jp}_{jj}")
            for i in range(NPAIR):
                nc.tensor.matmul(cs, ones128, E[:, 2 * i:2 * i + 2, jj, :],
                                 start=(i == 0), stop=False, perf_mode=DR)
            nc.tensor.matmul(cs, ones128[:MTAIL, 0, :],
                             E[:MTAIL, NMC - 1, jj, :],
                             start=False, stop=True)
            rs = sbm.tile([P, NW], F32, tag="rs", bufs=4,
                          name=f"rs_{mat}_{jp}_{jj}")
            nc.vector.reciprocal_approx_fast(rs, cs)
            if mat == 3:
                nc.vector.tensor_scalar_mul(rs, rs, FACTOR)
            rss.append(rs)
        for cc in range(CC):
            csl = slice(cc * P, (cc + 1) * P)
            vp = ps.tile([P, 1024], F32, tag=tags[cc % 2], bufs=2,
                         name=f"vp_{mat}_{jp}_{cc}")
            for i in range(NPAIR):
                for jj in range(npj):
                    nc.tensor.matmul(vp[:, jj * 512:jj * 512 + NW],
                                     fT[:, 2 * i:2 * i + 2, csl],
                                     E[:, 2 * i:2 * i + 2, jj, :],
                                     start=(i == 0), stop=False, perf_mode=DR)
            for jj in range(npj):
                nc.tensor.matmul(vp[:, jj * 512:jj * 512 + NW],
                                 fT[:MTAIL, NMC - 1, csl],
                                 E[:MTAIL, NMC - 1, jj, :],
                                 start=False, stop=True)
            for jj in range(npj):
                vslice = vp[:, jj * 512:jj * 512 + NW]
                if mat == 2:
                    # park FACTOR * (f2 @ a2^T) in bf16 until the mat3 phase
                    t2 = sbm.tile([P, NW], BF16, tag="t2", bufs=28,
                                  name=f"t2_{jp}_{jj}_{cc}")
                    nc.vector.scalar_tensor_tensor(t2, vslice, FACTOR, rss[jj],
                                                   MUL, MUL)
                    T2[(jp, jj, cc)] = t2
                else:
                    js = jss[jj]
                    ta = sbm.tile([P, NW], F32, tag="t", bufs=4,
                                  name=f"ta_{jp}_{jj}_{cc}")
                    nc.vector.tensor_mul(ta, vslice, rss[jj])
                    if jp == NJP - 1:
                        eng = nc.vector if cc % 2 == 0 else nc.gpsimd
                    else:
                        eng = nc.gpsimd
                    tb = sbm.tile([P, NW], F32, tag="t", bufs=4,
                                  name=f"tb_{jp}_{jj}_{cc}")
                    eng.tensor_add(tb, ta, T2.pop((jp, jj, cc)))
                    fs = sbm.tile([P, NW], F32, tag="f1s", bufs=4,
                                  name=f"f1s_{jp}_{jj}_{cc}")
                    nc.sync.dma_start(fs, fm1[csl, js])
                    o = sbm.tile([P, NW], F32, tag="outs", bufs=4,
                                 name=f"o_{jp}_{jj}_{cc}")
                    eng.tensor_add(o, tb, fs)
                    nc.sync.dma_start(out_ap[csl, js], o)

    # ---- software pipeline: scores for the next pair are emitted (and thus
    # PE-prioritized) ahead of the value phase of the current pair ----
    emit_scores(2, 0, drip=drip2)
    emit_scores(2, 1)
    emit_scores(2, 2)
    values(2, 0)
    emit_scores(2, 3, drip=drip3)
    values(2, 1)
    _f3_norms(tc, sbP, sbm, ps, st, fsq3)
    emit_scores(3, 0)
    values(2, 2)
    emit_scores(3, 1)
    values(2, 3)
    emit_scores(3, 2)
    values(3, 0)
    emit_scores(3, 3)
    values(3, 1)
    values(3, 2)
    values(3, 3, steal_sp=True)


def _build():
    nc = bacc.Bacc("TRN2", target_bir_lowering=False, debug=False,
                   num_devices=B)
    fm1 = nc.dram_tensor("fm1", [C, HW], F32, kind="ExternalInput").ap()
    fm2 = nc.dram_tensor("fm2", [C, HW], F32, kind="ExternalInput").ap()
    fm3 = nc.dram_tensor("fm3", [C, HW], F32, kind="ExternalInput").ap()
    out = nc.dram_tensor("out", [C, HW], F32, kind="ExternalOutput").ap()

    with tile.TileContext(nc) as tc:
        with tc.tile_pool(name="sbP", bufs=1) as sbP, \
             tc.tile_pool(name="ps", bufs=1, space="PSUM") as ps:
            st = {}
            with tc.tile_pool(name="pre", bufs=1) as pre:
                _build_preproc(tc, sbP, pre, ps, fm1, fm2, st)
            with tc.tile_pool(name="sbm", bufs=1) as sbm:
                _build_main(tc, sbP, sbm, ps, out, fm1, fm3, st)
    nc.compile()
    return nc


def _get_nc():
    global _cached_nc
    if _cached_nc is None:
        _cached_nc = _build()
    return _cached_nc


def kernel(**inputs):
    fm1 = np.ascontiguousarray(
        np.asarray(inputs["fm1"], dtype=np.float32).reshape(B, C, HW))
    fm2 = np.ascontiguousarray(
        np.asarray(inputs["fm2"], dtype=np.float32).reshape(B, C, HW))
    fm3 = np.ascontiguousarray(
        np.asarray(inputs["fm3"], dtype=np.float32).reshape(B, C, HW))

    nc = _get_nc()
    in_maps = [{"fm1": fm1[b], "fm2": fm2[b], "fm3": fm3[b]} for b in range(B)]
    res = run_bass_kernel_spmd(nc, in_maps, core_ids=list(range(B)),
                               trace=TRACE)
    kernel.last_results = res
    out = np.stack([res.results[b]["out"] for b in range(B)])
    return out.reshape(B, C, H, W).astype(np.float32)


kernel.last_results = None


if __name__ == "__main__":
    rng = np.random.default_rng(0)
    ins = {k: rng.standard_normal((B, C, H, W)).astype(np.float32)
           for k in ("fm1", "fm2", "fm3")}
    o = kernel(**ins)
    print("out shape", o.shape, o.dtype)
